# revision 3
# baseline (speedup 1.0000x reference)
"""AllPoleDigitalFilter Trainium2 kernel — segmented block-solve version.

y[t] = K_int[t]*x[t] - sum_{i=1..30} a_int[t,i] * y[t-i]
with a_int/K_int linearly interpolated from frame coefficients (period 80).

Strategy (per core, 8 of 64 batch sequences):
 - Overlap-save chunking: 16 chunks of L=1000 per sequence, W=80-sample
   warmup recomputed from zero state (homogeneous response decays to
   ~3e-4 of initial magnitude in 80 samples for these coefficients).
 - 128 partitions = 128 chunk instances (2 parities x 8 seqs x 8 chunks),
   window = 1080 samples each.
 - The order-30 recurrence advances S=8 samples per 5 Vector-engine
   instructions (instead of 2 instructions per sample):
     prod1[k,d] = afull[t0+k, d] * ybuf[t0+k+d]   (d=0..29, lag 30-d;
                  in-segment ybuf slots are still zero, so in-segment
                  taps contribute nothing)
     F[k]  = reduce_X(prod1)                       (known-history part)
     b[k]  = xgf[t0+k] - F[k]
     prod2 = E[seg] * b  (broadcast)               (8x8 matvec)
     y_seg = reduce_X(prod2) -> ybuf[30+t0 : +8]
   where E[seg] = (I + N_seg)^{-1}, N_seg the strictly-lower in-segment
   coefficient matrix; E is precomputed on-device per segment via a
   7-step forward substitution (rect-multiply + negated reduce).
 - Coefficient interpolation runs OFF the Vector engine: the Pool engine
   computes frac*dfh with double-broadcast APs, and the frame term
   arrives via gpsimd accumulate-DMA from a host-side replicated gather
   (pure layout). The gain channel xgf is assembled the same way.
"""
import numpy as np

B, T = 64, 16000
NSEQ = 8           # sequences per core
NCORE = 8
W = 80             # warmup samples per chunk
L = 1000           # chunk payload
WP = W + L         # window samples per instance (1080)
S = 8              # segment length
NSEG = WP // S     # 135
NU = WP // 40      # 27 half-frames per window
XP_LEN = W + T     # 16080

# afull-assembly blocks in half-frames (must sum to NU)
BLK_U = [5, 5, 5, 5, 5, 2]

_prog = None


def _build_program():
    import concourse.bacc as bacc
    import concourse.mybir as mybir
    import concourse.bass as bass
    from concourse.tile import TileContext

    f32 = mybir.dt.float32
    AP = bass.AP
    mult = mybir.AluOpType.mult
    add = mybir.AluOpType.add
    sub = mybir.AluOpType.subtract
    AXX = mybir.AxisListType.X

    nc = bacc.Bacc("TRN2", target_bir_lowering=False, name="apdf2",
                   detect_race_conditions=False)
    xp_d = nc.dram_tensor("xp", (NSEQ, XP_LEN), f32, kind="ExternalInput")
    frhr_d = nc.dram_tensor("frhr", (128, NU, 30), f32, kind="ExternalInput")
    frh1r_d = nc.dram_tensor("frh1r", (128, NU, 30), f32, kind="ExternalInput")
    kfr_d = nc.dram_tensor("kfr", (128, NU), f32, kind="ExternalInput")
    kfr1_d = nc.dram_tensor("kfr1", (128, NU), f32, kind="ExternalInput")
    ftab_d = nc.dram_tensor("ftab", (128, WP), f32, kind="ExternalInput")
    frhrep_d = nc.dram_tensor("frhrep", (128, WP, 30), f32, kind="ExternalInput")
    krep_d = nc.dram_tensor("krep", (128, WP), f32, kind="ExternalInput")
    y_d = nc.dram_tensor("y", (NSEQ, T), f32, kind="ExternalOutput")

    with TileContext(nc) as tc:
        with tc.tile_pool(name="sbuf", bufs=1) as pool:
            afull = pool.tile([128, WP, 30], f32)      # 129.6 KB/part
            E = pool.tile([128, NSEG, 64], f32)        # 34.6 KB
            escr = pool.tile([128, 25 * 49], f32)      # 4.9 KB
            ybuf = pool.tile([128, 30 + WP], f32)      # 4.44 KB
            xwin = pool.tile([128, WP], f32)
            xgf = pool.tile([128, WP], f32)
            kt = pool.tile([128, WP], f32)
            ftab = pool.tile([128, WP], f32)
            frhr = pool.tile([128, NU, 30], f32)
            frh1r = pool.tile([128, NU, 30], f32)
            dfhr = pool.tile([128, NU, 30], f32)
            kfr = pool.tile([128, NU], f32)
            kfr1 = pool.tile([128, NU], f32)
            dk = pool.tile([128, NU], f32)
            prod1 = pool.tile([128, S, 30], f32)
            prod2 = pool.tile([128, S, S], f32)
            fred = pool.tile([128, S], f32)
            bseg = pool.tile([128, S], f32)

            def tap(t, ap):
                """Manual AP over a tile's storage (element strides)."""
                base = t[:]
                return AP(tensor=base.tensor, offset=ap[0], ap=ap[1])

            AF = WP * 30     # afull partition stride
            ES = NSEG * 64   # E partition stride

            # ---------------- input DMAs ----------------
            nc.sync.dma_start(out=ftab[:], in_=ftab_d[:])
            nc.sync.dma_start(out=frhr[:].rearrange("p u d -> p (u d)"),
                              in_=frhr_d[:].rearrange("p u d -> p (u d)"))
            nc.sync.dma_start(out=frh1r[:].rearrange("p u d -> p (u d)"),
                              in_=frh1r_d[:].rearrange("p u d -> p (u d)"))
            nc.scalar.dma_start(out=kfr[:], in_=kfr_d[:])
            nc.scalar.dma_start(out=kfr1[:], in_=kfr1_d[:])

            # x windows: partition (parity, s, k) <- xp[s, 1000*(2k+par) : +WP]
            xw4 = xwin[:].rearrange("(c s k) j -> c s k j", c=2, s=8, k=8)
            for par in (0, 1):
                for s in range(NSEQ):
                    xsrc = AP(tensor=xp_d, offset=s * XP_LEN + 1000 * par,
                              ap=[[2000, 8], [1, WP]])
                    eng = nc.scalar if par == 0 else nc.sync
                    eng.dma_start(out=xw4[par, s], in_=xsrc)

            # ---------------- init ----------------
            nc.gpsimd.memset(ybuf[:], 0.0)
            nc.gpsimd.memset(E[:].rearrange("p s e -> p (s e)"), 0.0)
            # E diagonal = 1
            nc.gpsimd.memset(
                tap(E, (0, [[ES, 128], [64, NSEG], [9, 8]])), 1.0)

            # deltas
            nc.vector.tensor_tensor(
                out=dfhr[:].rearrange("p u d -> p (u d)"),
                in0=frh1r[:].rearrange("p u d -> p (u d)"),
                in1=frhr[:].rearrange("p u d -> p (u d)"), op=sub)
            nc.vector.tensor_tensor(out=dk[:], in0=kfr1[:], in1=kfr[:], op=sub)

            # ---------------- per-block pipeline ----------------
            u0 = 0
            s0 = 0
            out_slab = 0
            for blki, ublk in enumerate(BLK_U):
                c0 = u0 * 40
                cn = ublk * 40          # samples in block
                segb = cn // S          # segments in block

                # interp: afull[:, c0:c0+cn, :] = dfhr(u) * ftab(j)
                nc.gpsimd.tensor_tensor(
                    out=tap(afull, (c0 * 30,
                            [[AF, 128], [1200, ublk], [30, 40], [1, 30]])),
                    in0=tap(dfhr, (u0 * 30,
                            [[NU * 30, 128], [30, ublk], [0, 40], [1, 30]])),
                    in1=tap(ftab, (c0,
                            [[WP, 128], [40, ublk], [1, 40], [0, 30]])),
                    op=mult)
                # += frame term (replicated gather from DRAM). CCE accum
                # DMAs silently fail above ~2048 elems/partition -> chunk.
                CH = 1200
                for q0 in range(0, cn * 30, CH):
                    qn = min(CH, cn * 30 - q0)
                    nc.gpsimd.dma_start(
                        out=tap(afull, (c0 * 30 + q0, [[AF, 128], [1, qn]])),
                        in_=AP(tensor=frhrep_d, offset=c0 * 30 + q0,
                               ap=[[AF, 128], [1, qn]]),
                        accum_op=add)

                # gain channel for this block: kt = dk(u)*ftab; kt += krep;
                # xgf = kt * xwin
                nc.gpsimd.tensor_tensor(
                    out=tap(kt, (c0, [[WP, 128], [40, ublk], [1, 40]])),
                    in0=tap(dk, (u0, [[NU, 128], [1, ublk], [0, 40]])),
                    in1=tap(ftab, (c0, [[WP, 128], [40, ublk], [1, 40]])),
                    op=mult)
                nc.gpsimd.dma_start(
                    out=tap(kt, (c0, [[WP, 128], [1, cn]])),
                    in_=AP(tensor=krep_d, offset=c0, ap=[[WP, 128], [1, cn]]),
                    accum_op=add)
                nc.gpsimd.tensor_tensor(
                    out=tap(xgf, (c0, [[WP, 128], [1, cn]])),
                    in0=tap(kt, (c0, [[WP, 128], [1, cn]])),
                    in1=tap(xwin, (c0, [[WP, 128], [1, cn]])),
                    op=mult)

                # ---- E precompute for this block's segments (DVE) ----
                for kk in range(1, S):
                    # escr[p, sb, j, i] = a[t0+kk, lag i] * E[sb, kk-i, j]
                    nc.vector.tensor_tensor(
                        out=tap(escr, (0,
                                [[25 * 49, 128], [kk * kk, segb],
                                 [kk, kk], [1, kk]])),
                        in0=tap(afull, ((s0 * S + kk) * 30 + 29,
                                [[AF, 128], [240, segb], [0, kk], [-1, kk]])),
                        in1=tap(E, (s0 * 64 + (kk - 1) * 8,
                                [[ES, 128], [64, segb], [1, kk], [-8, kk]])),
                        op=mult)
                    # E[sb, kk, 0:kk] = -sum_i escr
                    nc.vector.tensor_reduce(
                        out=tap(E, (s0 * 64 + kk * 8,
                                [[ES, 128], [64, segb], [1, kk]])),
                        in_=tap(escr, (0,
                                [[25 * 49, 128], [kk * kk, segb],
                                 [kk, kk], [1, kk]])),
                        axis=AXX, op=add, negate=True)

                # ---- chain segments (DVE) ----
                for sl in range(segb):
                    seg = s0 + sl
                    t0 = seg * S
                    nc.vector.tensor_tensor(
                        out=prod1[:],
                        in0=afull[:, t0 : t0 + S, :],
                        in1=tap(ybuf, (t0, [[30 + WP, 128], [1, S], [1, 30]])),
                        op=mult)
                    nc.vector.tensor_reduce(
                        out=fred[:], in_=prod1[:], axis=AXX, op=add)
                    nc.vector.tensor_tensor(
                        out=bseg[:], in0=xgf[:, t0 : t0 + S], in1=fred[:],
                        op=sub)
                    nc.vector.tensor_tensor(
                        out=prod2[:],
                        in0=tap(E, (seg * 64, [[ES, 128], [8, 8], [1, 8]])),
                        in1=tap(bseg, (0, [[S, 128], [0, 8], [1, 8]])),
                        op=mult)
                    nc.vector.tensor_reduce(
                        out=ybuf[:, 30 + t0 : 30 + t0 + S],
                        in_=prod2[:].rearrange("p a b -> p a b"),
                        axis=AXX, op=add)

                u0 += ublk
                s0 += segb

                # first output slab once payload [0,500) is done (after
                # block 3: samples 0..800 cover payload up to 720)
                if blki == 3 and out_slab == 0:
                    out_slab = 1
                    yva = ybuf[:, 30 + W : 30 + W + 500].rearrange(
                        "(c s k) j -> c s k j", c=2, s=8, k=8)
                    for par in (0, 1):
                        for s in range(NSEQ):
                            dst = AP(tensor=y_d, offset=s * T + 1000 * par,
                                     ap=[[2000, 8], [1, 500]])
                            eng = nc.scalar if (s % 2 == 0) else nc.sync
                            eng.dma_start(out=dst, in_=yva[par, s])

            # ---------------- final output DMAs ----------------
            yv = ybuf[:, 30 + W + 500 : 30 + W + L].rearrange(
                "(c s k) j -> c s k j", c=2, s=8, k=8)
            for par in (0, 1):
                for s in range(NSEQ):
                    dst = AP(tensor=y_d, offset=s * T + 1000 * par + 500,
                             ap=[[2000, 8], [1, 500]])
                    eng = nc.scalar if (s % 2 == 0) else nc.sync
                    eng.dma_start(out=dst, in_=yv[par, s])

    nc.compile()
    return nc


def _get_prog():
    global _prog
    if _prog is None:
        _prog = _build_program()
    return _prog


def _host_inputs(x, a):
    x = np.ascontiguousarray(x, dtype=np.float32)
    a = np.ascontiguousarray(a, dtype=np.float32)
    xp = np.zeros((B, XP_LEN), np.float32)
    xp[:, W:] = x
    # replicate-padded frames per sequence: [B, 201, 31]
    af = np.concatenate([a, a[:, -1:, :]], axis=1)
    nfr = af.shape[1]  # 201
    # partition p = parity*64 + s*8 + k ; chunk m = 2k + parity
    par = np.arange(128) // 64
    sq = (np.arange(128) % 64) // 8
    kc = np.arange(128) % 8
    m = 2 * kc + par
    w0 = 1000 * m - W
    n0 = np.floor_divide(w0, 80)
    phi = w0 - 80 * n0              # 0 or 40
    u = np.arange(NU)
    nl = (40 * u[None, :] + phi[:, None]) // 80          # [128, NU]
    idx = np.clip(n0[:, None] + nl, 0, nfr - 1)
    idx1 = np.clip(n0[:, None] + nl + 1, 0, nfr - 1)
    jl = np.arange(WP)
    ftab = (((jl[None, :] + phi[:, None]) % 80) / 80.0).astype(np.float32)
    rev = 30 - np.arange(30)        # d -> coeff index 30-d (lag 30-d)
    in_maps = []
    for c in range(NCORE):
        sl = slice(c * NSEQ, (c + 1) * NSEQ)
        seqg = c * NSEQ + sq
        frhr = af[seqg[:, None, None], idx[:, :, None], rev[None, None, :]]
        frh1r = af[seqg[:, None, None], idx1[:, :, None], rev[None, None, :]]
        kfr = af[seqg[:, None], idx, 0]
        kfr1 = af[seqg[:, None], idx1, 0]
        in_maps.append({
            "xp": xp[sl],
            "frhr": np.ascontiguousarray(frhr, np.float32),
            "frh1r": np.ascontiguousarray(frh1r, np.float32),
            "kfr": np.ascontiguousarray(kfr, np.float32),
            "kfr1": np.ascontiguousarray(kfr1, np.float32),
            "ftab": ftab,
            "frhrep": np.ascontiguousarray(
                np.repeat(frhr, 40, axis=1), np.float32),
            "krep": np.ascontiguousarray(
                np.repeat(kfr, 40, axis=1), np.float32),
        })
    return in_maps


def kernel(x, a):
    from concourse import bass_utils

    nc = _get_prog()
    in_maps = _host_inputs(x, a)
    res = bass_utils.run_bass_kernel_spmd(nc, in_maps, core_ids=list(range(NCORE)))
    out = np.empty((B, T), np.float32)
    for c in range(NCORE):
        out[c * NSEQ : (c + 1) * NSEQ] = res.results[c]["y"]
    return out


# revision 4
# speedup vs baseline: 1.1520x; 1.1520x over previous
"""AllPoleDigitalFilter Trainium2 kernel — segmented block-solve, v3.

y[t] = K_int[t]*x[t] - sum_{i=1..30} a_int[t,i] * y[t-i]
with a_int/K_int linearly interpolated from frame coefficients (period 80).

Per core (8 of 64 sequences): overlap-save into 128 chunk instances
(2 parities x 8 seqs x 8 chunks) of 1080 samples (80-sample warmup from
zero state; homogeneous response decays ~3e-4 in 80 samples here).

The order-30 recurrence advances S=8 samples per 4 Vector instructions
using a custom DVE op CUMSUM_MUL (out = running cumsum of Src0*Src1):
  far:  cumsum over the [8 x 30] rectangle afull[t0+k,d]*ybuf[t0+k+d]
        (in-segment ybuf slots still zero => only known-history taps
        contribute); a 0-stride output AP keeps only each row's final
        cumsum -> fscr[1..8].
  b[k] = xgf[t0+k] - (fscr[k+1]-fscr[k])       (2 small tensor ops)
  near: cumsum over G[seg][8x8] * b[j] with G = row-differenced
        E = (I+N)^{-1}; by telescoping, each row-final cumsum equals
        y[t0+k] and a 0-stride output writes ybuf directly.
E is precomputed per segment on-device (7-step forward substitution,
batched over segments), then differenced in place (descending rows).

Interpolation runs off the Vector engine: Pool computes frac*dfh with
double-broadcast APs into bf16 afull; the frame term is added by gpsimd
accumulate-DMA from a host-side replicated gather (pure layout), in
<=1800-element chunks (CCE accum silently fails above ~2048).
"""
import numpy as np

B, T = 64, 16000
NSEQ = 8           # sequences per core
NCORE = 8
W = 80             # warmup samples per chunk
L = 1000           # chunk payload
WP = W + L         # window samples per instance (1080)
S = 8              # segment length
NSEG = WP // S     # 135
NU = WP // 40      # 27 half-frames per window
XP_LEN = W + T     # 16080

BLK_U = [2, 3, 5, 5, 5, 5, 2]   # afull-assembly blocks (half-frames)
ACH = 1800                       # accum-DMA chunk (elements)

_prog = None
_cm_op = None


def _register_cumsum_mul():
    """Append the CUMSUM_MUL op to the custom-DVE registry (documented
    extension point; per-NEFF table, existing ops untouched)."""
    global _cm_op
    if _cm_op is not None:
        return _cm_op
    from concourse.dve_spec import Spec, Src0, Src1, scan, AluOp, lower, _has_src1
    from concourse.dve_uop import DveOpSpec
    from concourse.dve_ops import DveOp, OPS, _SUB_OPCODE_FOR_NAME, \
        _CUSTOM_DVE_ROW_BASE

    if "CUMSUM_MUL_APDF" in _SUB_OPCODE_FOR_NAME:
        _cm_op = next(o for o in OPS if o.name == "CUMSUM_MUL_APDF")
        return _cm_op
    spec = Spec(
        body=scan(AluOp.ADD, Src0 * Src1),
        reference=lambda in0, in1, c0, c1, c2: np.cumsum(
            in0.astype(np.float32) * in1.astype(np.float32), axis=-1),
    )
    shas = {}
    for ver in ("v3", "v4"):
        s = DveOpSpec(name="CUMSUM_MUL_APDF", opcode=0,
                      uops=lower(spec, ver=ver), rd1_en=_has_src1(spec))
        shas[ver] = s.sha(ver)
    op = DveOp("CUMSUM_MUL_APDF", spec, subdim=False, uops_sha=shas)
    OPS.append(op)
    _SUB_OPCODE_FOR_NAME[op.name] = _CUSTOM_DVE_ROW_BASE + len(OPS) - 1
    _cm_op = op
    return op


def _build_program():
    import concourse.bacc as bacc
    import concourse.mybir as mybir
    import concourse.bass as bass
    from concourse.tile import TileContext

    CM = _register_cumsum_mul()

    f32 = mybir.dt.float32
    bf16 = mybir.dt.bfloat16
    AP = bass.AP
    mult = mybir.AluOpType.mult
    add = mybir.AluOpType.add
    sub = mybir.AluOpType.subtract
    AXX = mybir.AxisListType.X

    nc = bacc.Bacc("TRN2", target_bir_lowering=False, name="apdf3",
                   detect_race_conditions=False)
    xp_d = nc.dram_tensor("xp", (NSEQ, XP_LEN), f32, kind="ExternalInput")
    frhr_d = nc.dram_tensor("frhr", (128, NU, 30), f32, kind="ExternalInput")
    frh1r_d = nc.dram_tensor("frh1r", (128, NU, 30), f32, kind="ExternalInput")
    kfr_d = nc.dram_tensor("kfr", (128, NU), f32, kind="ExternalInput")
    kfr1_d = nc.dram_tensor("kfr1", (128, NU), f32, kind="ExternalInput")
    ftab_d = nc.dram_tensor("ftab", (128, WP), f32, kind="ExternalInput")
    frhrep_d = nc.dram_tensor("frhrep", (128, WP, 30), bf16,
                              kind="ExternalInput")
    krep_d = nc.dram_tensor("krep", (128, WP), f32, kind="ExternalInput")
    y_d = nc.dram_tensor("y", (NSEQ, T), f32, kind="ExternalOutput")

    with TileContext(nc) as tc:
        with tc.tile_pool(name="sbuf", bufs=1) as pool:
            afull = pool.tile([128, WP, 30], bf16)     # 64.8 KB/part
            E = pool.tile([128, NSEG, 64], f32)        # 34.6 KB
            escr = pool.tile([128, 25 * 49], f32)      # 4.9 KB
            ybuf = pool.tile([128, 30 + WP], f32)
            xwin = pool.tile([128, WP], f32)
            xgf = pool.tile([128, WP], f32)
            kt = pool.tile([128, WP], f32)
            krepb = pool.tile([128, WP], f32)
            ftab = pool.tile([128, WP], f32)
            frhr = pool.tile([128, NU, 30], f32)
            frh1r = pool.tile([128, NU, 30], f32)
            dfhr = pool.tile([128, NU, 30], f32)
            kfr = pool.tile([128, NU], f32)
            kfr1 = pool.tile([128, NU], f32)
            dk = pool.tile([128, NU], f32)
            fscr = pool.tile([128, 9], f32)
            bseg = pool.tile([128, S], f32)

            def tap(t, off, apl):
                base = t[:]
                return AP(tensor=base.tensor, offset=off, ap=apl)

            AF = WP * 30     # afull partition stride (elements)
            ES = NSEG * 64   # E partition stride
            YS = 30 + WP     # ybuf partition stride

            # ---------------- input DMAs ----------------
            nc.sync.dma_start(out=ftab[:], in_=ftab_d[:])
            nc.sync.dma_start(out=frhr[:].rearrange("p u d -> p (u d)"),
                              in_=frhr_d[:].rearrange("p u d -> p (u d)"))
            nc.sync.dma_start(out=frh1r[:].rearrange("p u d -> p (u d)"),
                              in_=frh1r_d[:].rearrange("p u d -> p (u d)"))
            nc.scalar.dma_start(out=kfr[:], in_=kfr_d[:])
            nc.scalar.dma_start(out=kfr1[:], in_=kfr1_d[:])
            nc.scalar.dma_start(out=krepb[:], in_=krep_d[:])

            xw4 = xwin[:].rearrange("(c s k) j -> c s k j", c=2, s=8, k=8)
            for par in (0, 1):
                for s in range(NSEQ):
                    xsrc = AP(tensor=xp_d, offset=s * XP_LEN + 1000 * par,
                              ap=[[2000, 8], [1, WP]])
                    eng = nc.scalar if par == 0 else nc.sync
                    eng.dma_start(out=xw4[par, s], in_=xsrc)

            # ---------------- init ----------------
            nc.gpsimd.memset(ybuf[:], 0.0)
            nc.gpsimd.memset(fscr[:], 0.0)
            nc.gpsimd.memset(E[:].rearrange("p s e -> p (s e)"), 0.0)
            nc.gpsimd.memset(
                tap(E, 0, [[ES, 128], [64, NSEG], [9, 8]]), 1.0)

            # deltas (DVE, small)
            nc.vector.tensor_tensor(
                out=dfhr[:].rearrange("p u d -> p (u d)"),
                in0=frh1r[:].rearrange("p u d -> p (u d)"),
                in1=frhr[:].rearrange("p u d -> p (u d)"), op=sub)
            nc.vector.tensor_tensor(out=dk[:], in0=kfr1[:], in1=kfr[:], op=sub)

            # gain channel, whole window (Pool):
            # kt = dk(u)*ftab ; kt += krepb ; xgf = kt * xwin
            nc.gpsimd.tensor_tensor(
                out=tap(kt, 0, [[WP, 128], [40, NU], [1, 40]]),
                in0=tap(dk, 0, [[NU, 128], [1, NU], [0, 40]]),
                in1=tap(ftab, 0, [[WP, 128], [40, NU], [1, 40]]),
                op=mult)
            nc.gpsimd.tensor_tensor(out=kt[:], in0=kt[:], in1=krepb[:], op=add)
            nc.gpsimd.tensor_tensor(out=xgf[:], in0=kt[:], in1=xwin[:], op=mult)

            # ---------------- per-block pipeline ----------------
            u0 = 0
            s0 = 0
            emitted_slab = False
            for blki, ublk in enumerate(BLK_U):
                c0 = u0 * 40
                cn = ublk * 40
                segb = cn // S

                # interp: afull[:, c0:c0+cn, :] = dfhr(u) * ftab(j)  (bf16)
                nc.gpsimd.tensor_tensor(
                    out=tap(afull, c0 * 30,
                            [[AF, 128], [1200, ublk], [30, 40], [1, 30]]),
                    in0=tap(dfhr, u0 * 30,
                            [[NU * 30, 128], [30, ublk], [0, 40], [1, 30]]),
                    in1=tap(ftab, c0,
                            [[WP, 128], [40, ublk], [1, 40], [0, 30]]),
                    op=mult)
                # += frame term (bf16 accum-DMA, chunked)
                for q0 in range(0, cn * 30, ACH):
                    qn = min(ACH, cn * 30 - q0)
                    nc.gpsimd.dma_start(
                        out=tap(afull, c0 * 30 + q0, [[AF, 128], [1, qn]]),
                        in_=AP(tensor=frhrep_d, offset=c0 * 30 + q0,
                               ap=[[AF, 128], [1, qn]]),
                        accum_op=add)

                # ---- E precompute for this block (DVE) ----
                for kk in range(1, S):
                    nc.vector.tensor_tensor(
                        out=tap(escr, 0,
                                [[25 * 49, 128], [kk * kk, segb],
                                 [kk, kk], [1, kk]]),
                        in0=tap(afull, (s0 * S + kk) * 30 + 29,
                                [[AF, 128], [240, segb], [0, kk], [-1, kk]]),
                        in1=tap(E, s0 * 64 + (kk - 1) * 8,
                                [[ES, 128], [64, segb], [1, kk], [-8, kk]]),
                        op=mult)
                    nc.vector.tensor_reduce(
                        out=tap(E, s0 * 64 + kk * 8,
                                [[ES, 128], [64, segb], [1, kk]]),
                        in_=tap(escr, 0,
                                [[25 * 49, 128], [kk * kk, segb],
                                 [kk, kk], [1, kk]]),
                        axis=AXX, op=add, negate=True)
                # G = row-diff(E) in place, descending rows (rank-4 TT)
                nc.vector.tensor_tensor(
                    out=tap(E, s0 * 64 + 56,
                            [[ES, 128], [64, segb], [-8, 7], [1, 8]]),
                    in0=tap(E, s0 * 64 + 56,
                            [[ES, 128], [64, segb], [-8, 7], [1, 8]]),
                    in1=tap(E, s0 * 64 + 48,
                            [[ES, 128], [64, segb], [-8, 7], [1, 8]]),
                    op=sub)

                # ---- chain segments (DVE) ----
                for sl in range(segb):
                    seg = s0 + sl
                    t0 = seg * S
                    # far: row-final cumsums -> fscr[1..8]
                    nc.vector._custom_dve(
                        CM,
                        out=tap(fscr, 1, [[9, 128], [1, S], [0, 30]]),
                        in0=tap(afull, t0 * 30,
                                [[AF, 128], [30, S], [1, 30]]),
                        in1=tap(ybuf, t0, [[YS, 128], [1, S], [1, 30]]),
                    )
                    # b = xgf - (fscr[k+1] - fscr[k])
                    nc.vector.tensor_tensor(
                        out=bseg[:], in0=xgf[:, t0 : t0 + S],
                        in1=fscr[:, 1 : 1 + S], op=sub)
                    nc.vector.tensor_tensor(
                        out=bseg[:], in0=bseg[:], in1=fscr[:, 0:S], op=add)
                    # near: G[seg] x b, telescoped -> ybuf directly
                    nc.vector._custom_dve(
                        CM,
                        out=tap(ybuf, 30 + t0, [[YS, 128], [1, S], [0, S]]),
                        in0=tap(E, seg * 64, [[ES, 128], [8, S], [1, S]]),
                        in1=tap(bseg, 0, [[S, 128], [0, S], [1, S]]),
                    )

                u0 += ublk
                s0 += segb

                # first output slab once payload [0,500) is complete
                if s0 * S >= W + 500 + 30 and not emitted_slab:
                    emitted_slab = True
                    yva = ybuf[:, 30 + W : 30 + W + 500].rearrange(
                        "(c s k) j -> c s k j", c=2, s=8, k=8)
                    for par in (0, 1):
                        for s in range(NSEQ):
                            dst = AP(tensor=y_d, offset=s * T + 1000 * par,
                                     ap=[[2000, 8], [1, 500]])
                            eng = nc.scalar if (s % 2 == 0) else nc.sync
                            eng.dma_start(out=dst, in_=yva[par, s])

            # ---------------- final output DMAs ----------------
            yv = ybuf[:, 30 + W + 500 : 30 + W + L].rearrange(
                "(c s k) j -> c s k j", c=2, s=8, k=8)
            for par in (0, 1):
                for s in range(NSEQ):
                    dst = AP(tensor=y_d, offset=s * T + 1000 * par + 500,
                             ap=[[2000, 8], [1, 500]])
                    eng = nc.scalar if (s % 2 == 0) else nc.sync
                    eng.dma_start(out=dst, in_=yv[par, s])

    nc.compile()
    return nc


def _get_prog():
    global _prog
    if _prog is None:
        _prog = _build_program()
    return _prog


def _host_inputs(x, a):
    import ml_dtypes

    x = np.ascontiguousarray(x, dtype=np.float32)
    a = np.ascontiguousarray(a, dtype=np.float32)
    xp = np.zeros((B, XP_LEN), np.float32)
    xp[:, W:] = x
    af = np.concatenate([a, a[:, -1:, :]], axis=1)   # [B, 201, 31]
    nfr = af.shape[1]
    par = np.arange(128) // 64
    sq = (np.arange(128) % 64) // 8
    kc = np.arange(128) % 8
    m = 2 * kc + par
    w0 = 1000 * m - W
    n0 = np.floor_divide(w0, 80)
    phi = w0 - 80 * n0              # 0 or 40
    u = np.arange(NU)
    nl = (40 * u[None, :] + phi[:, None]) // 80
    idx = np.clip(n0[:, None] + nl, 0, nfr - 1)
    idx1 = np.clip(n0[:, None] + nl + 1, 0, nfr - 1)
    jl = np.arange(WP)
    ftab = (((jl[None, :] + phi[:, None]) % 80) / 80.0).astype(np.float32)
    rev = 30 - np.arange(30)        # d -> coeff index 30-d (lag 30-d)
    in_maps = []
    for c in range(NCORE):
        sl = slice(c * NSEQ, (c + 1) * NSEQ)
        seqg = c * NSEQ + sq
        frhr = af[seqg[:, None, None], idx[:, :, None], rev[None, None, :]]
        frh1r = af[seqg[:, None, None], idx1[:, :, None], rev[None, None, :]]
        kfr = af[seqg[:, None], idx, 0]
        kfr1 = af[seqg[:, None], idx1, 0]
        frhrep = np.repeat(frhr, 40, axis=1).astype(ml_dtypes.bfloat16)
        in_maps.append({
            "xp": xp[sl],
            "frhr": np.ascontiguousarray(frhr, np.float32),
            "frh1r": np.ascontiguousarray(frh1r, np.float32),
            "kfr": np.ascontiguousarray(kfr, np.float32),
            "kfr1": np.ascontiguousarray(kfr1, np.float32),
            "ftab": ftab,
            "frhrep": np.ascontiguousarray(frhrep).view(np.uint16),
            "krep": np.ascontiguousarray(
                np.repeat(kfr, 40, axis=1), np.float32),
        })
    return in_maps


def kernel(x, a):
    from concourse import bass_utils

    nc = _get_prog()
    in_maps = _host_inputs(x, a)
    res = bass_utils.run_bass_kernel_spmd(nc, in_maps, core_ids=list(range(NCORE)))
    out = np.empty((B, T), np.float32)
    for c in range(NCORE):
        out[c * NSEQ : (c + 1) * NSEQ] = res.results[c]["y"]
    return out
